# revision 23
# baseline (speedup 1.0000x reference)
"""Multi-head cross attention on 8 Trainium2 NeuronCores.

Sharding: core c = b*4 + g handles batch b (of 2) and head-group g (4 heads
of the 16).  Each core projects Q/K/V for its 4 heads, runs attention, and
computes a partial output projection with its 256 rows of Wo; the host sums
the 4 partials per batch (plus bo and the bv@Wo term, exact because softmax
rows sum to 1).

v2: single interleaved instruction stream built around the ACT engine's exp
throughput floor (128 exp instrs x (1024+352)/1.2GHz ~= 147us/core):
  - minimal preamble (KT pair0 -> QT(pair0,qq0) -> first scores) so exp
    starts as soon as the DMA-critical 6MB (xkv + wk/wq + xq-qq0) lands;
  - every other matmul (V', KT pair1, remaining QT, out-proj) is emitted as
    "filler" work inside the attention j-loop's PE slack (ACT needs 1147ns
    per kv-tile, attention matmuls only ~640ns);
  - per-slot emission order is scores(j) -> exp(j) -> fillers -> AV(j-1) so
    the PE's wait-on-exp never sits in front of the next scores;
  - AVs may lag exp by up to the p_t ring (bufs=12) which absorbs the
    V'-heavy first block;
  - normalize is 1/rowsum via reciprocal_approx_fast (5x faster than DVE
    RECIPROCAL) + gpsimd partition_broadcast + one tensor_tensor that reads
    the AV PSUM directly;
  - Q/K bias adds ride on DVE (tensor_scalar_add), ACT does exp only;
  - out partials are written bf16 (half the DMA), host gathers in fp32.
Dataflow is fully "transposed" so no on-device transposes are needed:
  - QT[dh, sq]  = Wq_g.T @ xqT,  KT[dh, skv] = Wk_g.T @ xkvT (two heads
    packed per 128-partition tile)
  - V[skv, dh]  = xkvT.T @ Wv'_g (Wv' has a zero column after each head,
    memset to ones -> fused softmax row-sums)
  - S^T[skv, q] = KT_h.T @ QT_h  (K=64; the two heads of a pair use
    disjoint PE row groups and execute concurrently)
  - P^T = exp(S^T / 8)           (no max subtraction; |scores/8| < ~3)
  - O'^T = [V_h|1|...].T @ P^T   (128-wide lhsT window; row 64 = rowsum)
  - O^T = O'^T[0:64] * (1/rowsum)
  - out_partial[sq, 1024] = O^T_allheads.T @ Wo_g
Matmuls run in bf16 (fp32 PSUM accumulation; measured rel err ~4e-3).
"""

import sys

sys.path.insert(0, "/opt/trn_rl_repo")

import ml_dtypes
import numpy as np

BF16NP = ml_dtypes.bfloat16

B, SQ, SKV, D, H = 2, 2048, 2048, 1024, 16
DH = D // H          # 64
N_CORES = 8
G = 4                # head groups
HPG = H // G         # heads per group = 4
GC = HPG * DH        # group width = 256

_nc_cache = None


def _build_nc():
    import concourse.mybir as mybir
    import concourse.tile as tile
    from concourse import bacc

    F32 = mybir.dt.float32
    BF16 = mybir.dt.bfloat16
    AF = mybir.ActivationFunctionType
    MUL = mybir.AluOpType.mult

    nc = bacc.Bacc("TRN2", target_bir_lowering=False, debug=False,
                   num_devices=N_CORES)

    # all inputs host-pre-tiled so every DMA reads >=4KB contiguous per
    # SBUF partition (small descriptors run the queues at a fraction of
    # their rate)
    xqT_d = nc.dram_tensor("xqT", [128, 4, D // 128, 512], BF16,
                           kind="ExternalInput").ap()
    xkvT_d = nc.dram_tensor("xkvT", [128, 4, D // 128, 512], BF16,
                            kind="ExternalInput").ap()
    wq_d = nc.dram_tensor("wq", [128, 2, D // 128, 128], BF16,
                          kind="ExternalInput").ap()
    wk_d = nc.dram_tensor("wk", [128, 2, D // 128, 128], BF16,
                          kind="ExternalInput").ap()
    # Wv' with a zero column after each head's 64 (memset to ones on device)
    wvp_d = nc.dram_tensor("wvp", [128, D // 128, HPG * 65], BF16,
                           kind="ExternalInput").ap()
    wo_d = nc.dram_tensor("wo", [128, 2, D], BF16, kind="ExternalInput").ap()
    bq_d = nc.dram_tensor("bq2", [128, 2], F32, kind="ExternalInput").ap()
    bk_d = nc.dram_tensor("bk2", [128, 2], F32, kind="ExternalInput").ap()
    out_d = nc.dram_tensor("out_p", [SQ, D], BF16, kind="ExternalOutput").ap()

    ND = D // 128        # 8 d-tiles (contraction over D)
    NJ = SKV // 128      # 16 kv tiles
    VW = HPG * 65        # 260, V' row width
    scale = 1.0 / float(np.sqrt(DH))

    with tile.TileContext(nc) as tc:
        with (
            tc.tile_pool(name="persist", bufs=1) as pp,
            tc.tile_pool(name="ring", bufs=1) as rg,
        ):
            # ---- persistent SBUF tiles --------------------------------
            qt_sb = pp.tile([128, 2 * SQ], BF16, tag="qt_sb")
            kt_sb = pp.tile([128, 2 * SKV], BF16, tag="kt_sb")
            vp_sb = pp.tile([128, NJ * VW + 63], BF16, tag="vp_sb")
            o_sbA = pp.tile([128, 2 * 1024], BF16, tag="o_sbA")
            o_sbB = pp.tile([128, 2 * 1024], BF16, tag="o_sbB")
            bq_sb = pp.tile([128, 2], F32, tag="bq_sb")
            bk_sb = pp.tile([128, 2], F32, tag="bk_sb")
            wk_sb = pp.tile([128, ND * GC], BF16, tag="wk_sb")
            wq_sb = pp.tile([128, ND * GC], BF16, tag="wq_sb")
            wvp_sb = pp.tile([128, ND * VW], BF16, tag="wvp_sb")
            wo_sb = pp.tile([128, 2 * D], BF16, tag="wo_sb")
            warm_sb = pp.tile([128, 2], F32, tag="warm_sb")
            # xkv per kv-column-quarter: [128, d, 512] so the first scores
            # and V' tiles only gate on 1MB of x, not 4MB
            xkvC = [pp.tile([128, ND, 512], BF16, tag=f"xkvC{qc}",
                            name=f"xkvC{qc}") for qc in range(4)]
            # xq per q-quarter: [128, d, 512]
            xq = [pp.tile([128, ND, 512], BF16, tag=f"xq{qq}", name=f"xq{qq}")
                  for qq in range(4)]

            # ---- DMA, split over the 3 DMA-capable queues (sync + scalar
            # are HWDGE, gpsimd is SWDGE), each in first-need order.
            # exp0 gates on wk-p0+xkvC0 (sync) and wq-p0+xq0 (scalar); keep
            # those queues short so the critical 2.5MB lands first.
            def wsl(w_sb, p):
                return w_sb[:, p * ND * 128:(p + 1) * ND * 128].rearrange(
                    "p (d n) -> p d n", d=ND)
            # critical set (gates exp0): wk-p0 + xkvC0 on sync, wq-p0 +
            # xq0 on scalar, only tiny/soon-needed items early on gpsimd;
            # everything else queues BEHIND so it cannot steal bandwidth
            nc.sync.dma_start(out=bk_sb[:], in_=bk_d[:])
            nc.sync.dma_start(out=wsl(wk_sb, 0), in_=wk_d[:, 0, :, :])
            nc.sync.dma_start(out=xkvC[0][:, 0:4, :], in_=xkvT_d[:, 0, 0:4, :])
            nc.sync.dma_start(out=xkvC[0][:, 4:8, :], in_=xkvT_d[:, 0, 4:8, :])
            nc.scalar.dma_start(out=wsl(wq_sb, 0), in_=wq_d[:, 0, :, :])
            nc.scalar.dma_start(out=xq[0][:, 0:4, :], in_=xqT_d[:, 0, 0:4, :])
            nc.scalar.dma_start(out=xq[0][:, 4:8, :], in_=xqT_d[:, 0, 4:8, :])
            nc.gpsimd.dma_start(out=bq_sb[:], in_=bq_d[:])
            nc.gpsimd.dma_start(
                out=wvp_sb[:].rearrange("p (d n) -> p d n", d=ND),
                in_=wvp_d[:])
            # xkv tail, deadline-ordered
            nc.sync.dma_start(out=xkvC[1][:], in_=xkvT_d[:, 1, :, :])
            nc.scalar.dma_start(out=xkvC[2][:], in_=xkvT_d[:, 2, :, :])
            nc.sync.dma_start(out=xkvC[3][:], in_=xkvT_d[:, 3, :, :])
            # the rest
            nc.gpsimd.dma_start(out=xq[1][:], in_=xqT_d[:, 1, :, :])
            nc.scalar.dma_start(out=xq[2][:], in_=xqT_d[:, 2, :, :])
            nc.scalar.dma_start(out=wsl(wq_sb, 1), in_=wq_d[:, 1, :, :])
            nc.sync.dma_start(out=wsl(wk_sb, 1), in_=wk_d[:, 1, :, :])
            nc.gpsimd.dma_start(
                out=wo_sb[:].rearrange("p (t n) -> p t n", t=2),
                in_=wo_d[:])
            nc.gpsimd.dma_start(out=xq[3][:], in_=xqT_d[:, 3, :, :])

            # load the exp table set during the preamble (first ACT call to a
            # new set costs ~2.7us; park it off the critical path)
            nc.scalar.activation(warm_sb[:], bq_sb[:], AF.Exp)
            # zero the V' tail pad and set every ones-column once up front;
            # the V' copies below never touch the ones columns, so no
            # write-after-write ordering is needed
            nc.gpsimd.memset(vp_sb[:, NJ * VW:NJ * VW + 63], 0.0)
            nc.gpsimd.memset(vp_sb[:, 64:NJ * VW:65], 1.0)

            # ---- preamble PSUM: KT(pair0, chunk0) + QT(p0, qq0) -------
            with tc.tile_pool(name="psPre", bufs=1, space="PSUM") as psA:
                pk0 = psA.tile([128, 512], F32, tag="pre", bufs=2, name="pk0")
                pq0 = psA.tile([128, 512], F32, tag="pre", bufs=2, name="pq0")
                for half in range(2):
                    for d in range(half * 4, half * 4 + 4):
                        nc.tensor.matmul(
                            pk0[:],
                            wk_sb[:, d * 128:(d + 1) * 128],
                            xkvC[0][:, d, :],
                            start=(d == 0), stop=(d == ND - 1),
                        )
                    for d in range(half * 4, half * 4 + 4):
                        nc.tensor.matmul(
                            pq0[:],
                            wq_sb[:, d * 128:(d + 1) * 128],
                            xq[0][:, d, :],
                            start=(d == 0), stop=(d == ND - 1),
                        )
                nc.vector.tensor_scalar_add(
                    kt_sb[:, 0:512], pk0[:], bk_sb[:, 0:1])
                nc.vector.tensor_scalar_add(
                    qt_sb[:, 0:512], pq0[:], bq_sb[:, 0:1])

            # ---- attention + fillers ----------------------------------
            with (
                tc.tile_pool(name="at", bufs=1) as at,
                tc.tile_pool(name="psAt", bufs=1, space="PSUM") as ps,
            ):
                # ------ filler closures --------------------------------
                def mk_vprime(j):
                    def emit():
                        pv = ps.tile([128, 512], F32, tag="fill", bufs=2,
                                     name=f"pv{j}")
                        qc, jr = divmod(j, 4)
                        for d in range(ND):
                            nc.tensor.matmul(
                                pv[:, 0:VW],
                                xkvC[qc][:, d, jr * 128:(jr + 1) * 128],
                                wvp_sb[:, d * VW:(d + 1) * VW],
                                start=(d == 0), stop=(d == ND - 1),
                            )
                        for h in range(HPG):
                            nc.vector.tensor_copy(
                                vp_sb[:, j * VW + h * 65:j * VW + h * 65 + 64],
                                pv[:, h * 65:h * 65 + 64])
                    return emit

                def mk_kt(p, qc):
                    def emit():
                        acc = ps.tile([128, 512], F32, tag="fill", bufs=2,
                                      name=f"ktf{p}{qc}")
                        for d in range(ND):
                            nc.tensor.matmul(
                                acc[:],
                                wk_sb[:, (p * ND + d) * 128:
                                      (p * ND + d + 1) * 128],
                                xkvC[qc][:, d, :],
                                start=(d == 0), stop=(d == ND - 1),
                            )
                        nc.vector.tensor_scalar_add(
                            kt_sb[:, p * SKV + qc * 512:p * SKV + (qc + 1) * 512],
                            acc[:], bk_sb[:, p:p + 1])
                    return emit

                def mk_qt(p, qq):
                    def emit():
                        acc = ps.tile([128, 512], F32, tag="fill", bufs=2,
                                      name=f"qtf{p}{qq}")
                        for d in range(ND):
                            nc.tensor.matmul(
                                acc[:],
                                wq_sb[:, (p * ND + d) * 128:
                                      (p * ND + d + 1) * 128],
                                xq[qq][:, d, :],
                                start=(d == 0), stop=(d == ND - 1),
                            )
                        nc.vector.tensor_scalar_add(
                            qt_sb[:, p * SQ + qq * 512:p * SQ + (qq + 1) * 512],
                            acc[:], bq_sb[:, p:p + 1])
                    return emit

                def mk_outproj(s):
                    def emit():
                        o_half = o_sbA if s < 8 else o_sbB
                        s8 = s % 8
                        ob = at.tile([128, 1024], BF16, tag="ob",
                                     bufs=3, name=f"ob{s}")
                        for n2 in range(2):
                            po = ps.tile([128, 512], F32, tag="fill", bufs=2,
                                         name=f"po{s}{n2}")
                            for tt in range(2):
                                nc.tensor.matmul(
                                    po[:],
                                    o_half[:, tt * 1024 + s8 * 128:
                                           tt * 1024 + (s8 + 1) * 128],
                                    wo_sb[:, tt * D + n2 * 512:
                                          tt * D + n2 * 512 + 512],
                                    start=(tt == 0), stop=(tt == 1),
                                )
                            nc.vector.tensor_copy(
                                ob[:, n2 * 512:(n2 + 1) * 512], po[:])
                        nc.sync.dma_start(
                            out=out_d[s * 128:(s + 1) * 128, :], in_=ob[:])
                    return emit

                # fillers per block, keyed by slot index (emission order)
                blocks = [(0, 0), (0, 1), (0, 2), (0, 3),
                          (1, 0), (1, 1), (1, 2), (1, 3)]
                fill = {b: {} for b in range(8)}

                def add_fill(b, slot, fn):
                    fill[b].setdefault(slot, []).append(fn)

                for j in range(NJ):
                    add_fill(0, j, mk_vprime(j))
                # remaining KT(p0) chunks just-in-time, paced by the xkv
                # column-block DMAs (chunk qc gates scores j>=4qc)
                add_fill(0, 2, mk_kt(0, 1))
                add_fill(0, 6, mk_kt(0, 2))
                add_fill(0, 10, mk_kt(0, 3))
                add_fill(0, 13, mk_qt(0, 1))
                # each QT/KT a full block ahead of its consumer so the
                # gpsimd bias-add is never on the critical path
                add_fill(1, 0, mk_qt(0, 2))
                add_fill(1, 8, mk_qt(0, 3))
                add_fill(2, 0, mk_kt(1, 0))
                add_fill(2, 4, mk_kt(1, 1))
                add_fill(2, 8, mk_kt(1, 2))
                add_fill(2, 12, mk_kt(1, 3))
                add_fill(3, 0, mk_qt(1, 0))
                add_fill(3, 4, mk_qt(1, 1))
                add_fill(3, 8, mk_qt(1, 2))
                add_fill(3, 12, mk_qt(1, 3))
                for i, s in enumerate(range(0, 4)):
                    add_fill(5, 2 + 3 * i, mk_outproj(s))
                for i, s in enumerate(range(4, 8)):
                    add_fill(6, 2 + 3 * i, mk_outproj(s))
                for i, s in enumerate(range(8, 12)):
                    add_fill(7, 1 + 2 * i, mk_outproj(s))

                def emit_norm(t, qq, hp, o_ps):
                    rs = at.tile([1, 512], F32, tag="rs", bufs=4,
                                 name=f"rs{t}{qq}{hp}")
                    nc.vector.tensor_copy(rs[:], o_ps[64:65, :])
                    rcp = at.tile([1, 512], F32, tag="rcp", bufs=4,
                                  name=f"rcp{t}{qq}{hp}")
                    nc.vector.reciprocal_approx_fast(rcp[:], rs[:])
                    bcs = at.tile([64, 512], F32, tag="bcs", bufs=4,
                                  name=f"bcs{t}{qq}{hp}")
                    nc.gpsimd.partition_broadcast(bcs[:], rcp[:], channels=64)
                    o_half = o_sbA if qq < 2 else o_sbB
                    col = t * 1024 + (qq % 2) * 512
                    nc.vector.tensor_tensor(
                        out=o_half[hp * 64:(hp + 1) * 64, col:col + 512],
                        in0=o_ps[0:64, :], in1=bcs[:], op=MUL)

                # ------ the attention stream ---------------------------
                for b, (t, qq) in enumerate(blocks):
                    o_ps = {}
                    for hp in range(2):
                        o_ps[hp] = ps.tile([128, 512], F32, tag="o_ps",
                                           bufs=2, name=f"o_ps{t}{qq}{hp}")
                    p_ts = {}
                    for j in range(NJ + 2):
                        if j < NJ:
                            st = ps.tile([128, 1024], F32, tag="st2", bufs=2,
                                         name=f"st{t}{qq}{j}")
                            for hp in range(2):
                                nc.tensor.matmul(
                                    st[:, hp * 512:(hp + 1) * 512],
                                    kt_sb[hp * 64:(hp + 1) * 64,
                                          t * SKV + j * 128:
                                          t * SKV + (j + 1) * 128],
                                    qt_sb[hp * 64:(hp + 1) * 64,
                                          t * SQ + qq * 512:
                                          t * SQ + (qq + 1) * 512],
                                    start=True, stop=True,
                                )
                            p_t = at.tile([128, 1024], BF16, tag="pt",
                                          bufs=12, name=f"pt{t}{qq}{j}")
                            nc.scalar.activation(p_t[:], st[:],
                                                 AF.Exp, scale=scale)
                            p_ts[j] = p_t
                        for fn in fill[b].get(j, []):
                            fn()
                        if j >= 2:
                            ja = j - 2
                            p_t = p_ts.pop(ja)
                            for hp in range(2):
                                h = 2 * t + hp
                                nc.tensor.matmul(
                                    o_ps[hp][:],
                                    vp_sb[:, ja * VW + h * 65:
                                          ja * VW + h * 65 + 128],
                                    p_t[:, hp * 512:(hp + 1) * 512],
                                    start=(ja == 0), stop=(ja == NJ - 1),
                                )
                    for hp in range(2):
                        emit_norm(t, qq, hp, o_ps[hp])

                # tail: last qq group's output projection
                for s in range(12, 16):
                    mk_outproj(s)()

    nc.compile()
    return nc


def build_in_maps(inputs):
    query_input = np.asarray(inputs["query_input"], dtype=np.float32)
    kv_input = np.asarray(inputs["kv_input"], dtype=np.float32)
    Wq = np.asarray(inputs["Wq"], dtype=np.float32)
    bq = np.asarray(inputs["bq"], dtype=np.float32)
    Wkv = np.asarray(inputs["Wkv"], dtype=np.float32)
    bkv = np.asarray(inputs["bkv"], dtype=np.float32)
    Wo = np.asarray(inputs["Wo"], dtype=np.float32)

    Wk = Wkv[:, :D]
    Wv = Wkv[:, D:]
    bk = bkv[:D]

    def tile_x(xt):
        # [D, SQ] -> [128, 4(qcol), 8(d), 512]
        return np.ascontiguousarray(
            xt.reshape(8, 128, 4, 512).transpose(1, 2, 0, 3)).astype(BF16NP)

    def tile_w(w):
        # [D, N] -> [128, 8(d), N]
        n = w.shape[1]
        return np.ascontiguousarray(
            w.reshape(8, 128, n).transpose(1, 0, 2)).astype(BF16NP)

    def tile_w_p(w):
        # [D, 256] -> [128, 2(pair), 8(d), 128]
        return np.ascontiguousarray(
            w.reshape(8, 128, 2, 128).transpose(1, 2, 0, 3)).astype(BF16NP)

    xT = [tile_x(query_input[b].T) for b in range(B)]
    kvT = [tile_x(kv_input[b].T) for b in range(B)]

    in_maps = []
    for c in range(N_CORES):
        b, g = divmod(c, G)
        c0 = g * GC
        wvp = np.zeros((D, HPG * 65), np.float32)
        for h in range(HPG):
                wvp[:, h * 65:h * 65 + 64] = Wv[:, c0 + h * DH:c0 + (h + 1) * DH]
        bq2 = bq[c0:c0 + GC].reshape(2, 128).T.copy()
        bk2 = bk[c0:c0 + GC].reshape(2, 128).T.copy()
        wo_t = np.ascontiguousarray(
            Wo[c0:c0 + GC, :].reshape(2, 128, D).transpose(1, 0, 2))
        in_maps.append({
                "xqT": xT[b],
                "xkvT": kvT[b],
                "wq": tile_w_p(Wq[:, c0:c0 + GC]),
                "wk": tile_w_p(Wk[:, c0:c0 + GC]),
                "wvp": tile_w(wvp),
                "wo": wo_t.astype(BF16NP),
                "bq2": np.ascontiguousarray(bq2),
                "bk2": np.ascontiguousarray(bk2),
        })
    return in_maps


def kernel(query_input, kv_input, Wq, bq, Wkv, bkv, Wo, bo):
    global _nc_cache
    from concourse import bass_utils

    if _nc_cache is None:
        _nc_cache = _build_nc()
    nc = _nc_cache

    Wkv = np.asarray(Wkv, dtype=np.float32)
    Wo = np.asarray(Wo, dtype=np.float32)
    bo = np.asarray(bo, dtype=np.float32)
    bv = np.asarray(bkv, np.float32)[D:]

    in_maps = build_in_maps(dict(
        query_input=query_input, kv_input=kv_input, Wq=Wq, bq=bq,
        Wkv=Wkv, bkv=bkv, Wo=Wo))

    res = bass_utils.run_bass_kernel_spmd(nc, in_maps,
                                          core_ids=list(range(N_CORES)))

    # gather: sum the 4 head-group partials per batch; add biases the device
    # left out (bo, and bv which passes through Wo since softmax rows sum to 1)
    tail = bv @ Wo + bo
    out = np.empty((B, SQ, D), np.float32)
    for b in range(B):
        acc = res.results[b * G + 0]["out_p"].astype(np.float32)
        for g in range(1, G):
                acc = acc + res.results[b * G + g]["out_p"].astype(np.float32)
        out[b] = acc + tail[None, :]
    return out


# revision 26
# speedup vs baseline: 1.1885x; 1.1885x over previous
"""Multi-head cross attention on 8 Trainium2 NeuronCores.

Sharding: core c = b*4 + g handles batch b (of 2) and head-group g (4 heads
of the 16).  Each core projects Q/K/V for its 4 heads, runs attention, and
computes a partial output projection with its 256 rows of Wo; the host sums
the 4 partials per batch (plus bo and the bv@Wo term, exact because softmax
rows sum to 1).

v2: single interleaved instruction stream built around the ACT engine's exp
throughput floor (128 exp instrs x (1024+352)/1.2GHz ~= 147us/core):
  - minimal preamble (KT pair0 -> QT(pair0,qq0) -> first scores) so exp
    starts as soon as the DMA-critical 6MB (xkv + wk/wq + xq-qq0) lands;
  - every other matmul (V', KT pair1, remaining QT, out-proj) is emitted as
    "filler" work inside the attention j-loop's PE slack (ACT needs 1147ns
    per kv-tile, attention matmuls only ~640ns);
  - per-slot emission order is scores(j) -> exp(j) -> fillers -> AV(j-1) so
    the PE's wait-on-exp never sits in front of the next scores;
  - AVs may lag exp by up to the p_t ring (bufs=12) which absorbs the
    V'-heavy first block;
  - normalize is 1/rowsum via reciprocal_approx_fast (5x faster than DVE
    RECIPROCAL) + gpsimd partition_broadcast + one tensor_tensor that reads
    the AV PSUM directly;
  - Q/K bias adds ride on DVE (tensor_scalar_add), ACT does exp only;
  - out partials are written bf16 (half the DMA), host gathers in fp32.
Dataflow is fully "transposed" so no on-device transposes are needed:
  - QT[dh, sq]  = Wq_g.T @ xqT,  KT[dh, skv] = Wk_g.T @ xkvT (two heads
    packed per 128-partition tile)
  - V[skv, dh]  = xkvT.T @ Wv'_g (Wv' has a zero column after each head,
    memset to ones -> fused softmax row-sums)
  - S^T[skv, q] = KT_h.T @ QT_h  (K=64; the two heads of a pair use
    disjoint PE row groups and execute concurrently)
  - P^T = exp(S^T / 8)           (no max subtraction; |scores/8| < ~3)
  - O'^T = [V_h|1|...].T @ P^T   (128-wide lhsT window; row 64 = rowsum)
  - O^T = O'^T[0:64] * (1/rowsum)
  - out_partial[sq, 1024] = O^T_allheads.T @ Wo_g
Matmuls run in bf16 (fp32 PSUM accumulation; measured rel err ~4e-3).
"""

import sys

sys.path.insert(0, "/opt/trn_rl_repo")

import ml_dtypes
import numpy as np

BF16NP = ml_dtypes.bfloat16

B, SQ, SKV, D, H = 2, 2048, 2048, 1024, 16
DH = D // H          # 64
N_CORES = 8
G = 4                # head groups
HPG = H // G         # heads per group = 4
GC = HPG * DH        # group width = 256

_nc_cache = None


def _build_nc():
    import concourse.mybir as mybir
    import concourse.tile as tile
    from concourse import bacc

    F32 = mybir.dt.float32
    BF16 = mybir.dt.bfloat16
    AF = mybir.ActivationFunctionType
    MUL = mybir.AluOpType.mult

    nc = bacc.Bacc("TRN2", target_bir_lowering=False, debug=False,
                   num_devices=N_CORES)

    # all inputs host-pre-tiled so every DMA reads >=4KB contiguous per
    # SBUF partition (small descriptors run the queues at a fraction of
    # their rate)
    xqT_d = nc.dram_tensor("xqT", [128, 4, D // 128, 512], BF16,
                           kind="ExternalInput").ap()
    xkvT_d = nc.dram_tensor("xkvT", [128, 4, D // 128, 512], BF16,
                            kind="ExternalInput").ap()
    wq_d = nc.dram_tensor("wq", [128, 2, D // 128, 128], BF16,
                          kind="ExternalInput").ap()
    wk_d = nc.dram_tensor("wk", [128, 2, D // 128, 128], BF16,
                          kind="ExternalInput").ap()
    # Wv' with a zero column after each head's 64 (memset to ones on device)
    wvp_d = nc.dram_tensor("wvp", [128, D // 128, HPG * 65], BF16,
                           kind="ExternalInput").ap()
    wo_d = nc.dram_tensor("wo", [128, 2, D], BF16, kind="ExternalInput").ap()
    bq_d = nc.dram_tensor("bq2", [128, 2], F32, kind="ExternalInput").ap()
    bk_d = nc.dram_tensor("bk2", [128, 2], F32, kind="ExternalInput").ap()
    out_d = nc.dram_tensor("out_p", [SQ, D], BF16, kind="ExternalOutput").ap()

    ND = D // 128        # 8 d-tiles (contraction over D)
    NJ = SKV // 128      # 16 kv tiles
    VW = HPG * 65        # 260, V' row width
    scale = 1.0 / float(np.sqrt(DH))

    with tile.TileContext(nc) as tc:
        with (
            tc.tile_pool(name="persist", bufs=1) as pp,
            tc.tile_pool(name="ring", bufs=1) as rg,
        ):
            # ---- persistent SBUF tiles --------------------------------
            qt_sb = pp.tile([128, 2 * SQ], BF16, tag="qt_sb")
            kt_sb = pp.tile([128, 2 * SKV], BF16, tag="kt_sb")
            vp_sb = pp.tile([128, NJ * VW + 63], BF16, tag="vp_sb")
            o_sbA = pp.tile([128, 2 * 1024], BF16, tag="o_sbA")
            o_sbB = pp.tile([128, 2 * 1024], BF16, tag="o_sbB")
            bq_sb = pp.tile([128, 2], F32, tag="bq_sb")
            bk_sb = pp.tile([128, 2], F32, tag="bk_sb")
            wk_sb = pp.tile([128, ND * GC], BF16, tag="wk_sb")
            wq_sb = pp.tile([128, ND * GC], BF16, tag="wq_sb")
            wvp_sb = pp.tile([128, ND * VW], BF16, tag="wvp_sb")
            wo_sb = pp.tile([128, 2 * D], BF16, tag="wo_sb")
            warm_sb = pp.tile([128, 2], F32, tag="warm_sb")
            # xkv per kv-column-quarter: [128, d, 512] so the first scores
            # and V' tiles only gate on 1MB of x, not 4MB
            xkvC = [pp.tile([128, ND, 512], BF16, tag=f"xkvC{qc}",
                            name=f"xkvC{qc}") for qc in range(4)]
            # xq per q-quarter: [128, d, 512]
            xq = [pp.tile([128, ND, 512], BF16, tag=f"xq{qq}", name=f"xq{qq}")
                  for qq in range(4)]

            # ---- DMA, split over the 3 DMA-capable queues (sync + scalar
            # are HWDGE, gpsimd is SWDGE), each in first-need order.
            # exp0 gates on wk-p0+xkvC0 (sync) and wq-p0+xq0 (scalar); keep
            # those queues short so the critical 2.5MB lands first.
            def wsl(w_sb, p):
                return w_sb[:, p * ND * 128:(p + 1) * ND * 128].rearrange(
                    "p (d n) -> p d n", d=ND)
            # critical set (gates exp0): wk-p0 + xkvC0 on sync, wq-p0 +
            # xq0 on scalar, only tiny/soon-needed items early on gpsimd;
            # everything else queues BEHIND so it cannot steal bandwidth
            nc.sync.dma_start(out=bk_sb[:], in_=bk_d[:])
            nc.sync.dma_start(out=wsl(wk_sb, 0), in_=wk_d[:, 0, :, :])
            nc.sync.dma_start(out=xkvC[0][:, 0:4, :], in_=xkvT_d[:, 0, 0:4, :])
            nc.sync.dma_start(out=xkvC[0][:, 4:8, :], in_=xkvT_d[:, 0, 4:8, :])
            nc.scalar.dma_start(out=wsl(wq_sb, 0), in_=wq_d[:, 0, :, :])
            nc.scalar.dma_start(out=xq[0][:, 0:4, :], in_=xqT_d[:, 0, 0:4, :])
            nc.scalar.dma_start(out=xq[0][:, 4:8, :], in_=xqT_d[:, 0, 4:8, :])
            nc.gpsimd.dma_start(out=bq_sb[:], in_=bq_d[:])
            nc.gpsimd.dma_start(
                out=wvp_sb[:].rearrange("p (d n) -> p d n", d=ND),
                in_=wvp_d[:])
            # xkv tail, deadline-ordered
            nc.sync.dma_start(out=xkvC[1][:], in_=xkvT_d[:, 1, :, :])
            nc.scalar.dma_start(out=xkvC[2][:], in_=xkvT_d[:, 2, :, :])
            nc.sync.dma_start(out=xkvC[3][:], in_=xkvT_d[:, 3, :, :])
            # the rest
            nc.gpsimd.dma_start(out=xq[1][:], in_=xqT_d[:, 1, :, :])
            nc.scalar.dma_start(out=xq[2][:], in_=xqT_d[:, 2, :, :])
            nc.scalar.dma_start(out=wsl(wq_sb, 1), in_=wq_d[:, 1, :, :])
            nc.sync.dma_start(out=wsl(wk_sb, 1), in_=wk_d[:, 1, :, :])
            nc.gpsimd.dma_start(
                out=wo_sb[:].rearrange("p (t n) -> p t n", t=2),
                in_=wo_d[:])
            nc.gpsimd.dma_start(out=xq[3][:], in_=xqT_d[:, 3, :, :])

            # load the exp table set during the preamble (first ACT call to a
            # new set costs ~2.7us; park it off the critical path)
            nc.scalar.activation(warm_sb[:], bq_sb[:], AF.Exp)
            # zero the V' tail pad and set every ones-column once up front;
            # the V' copies below never touch the ones columns, so no
            # write-after-write ordering is needed
            nc.gpsimd.memset(vp_sb[:, NJ * VW:NJ * VW + 63], 0.0)
            nc.gpsimd.memset(vp_sb[:, 64:NJ * VW:65], 1.0)

            # ---- preamble PSUM: KT(pair0, chunk0) + QT(p0, qq0) -------
            with tc.tile_pool(name="psPre", bufs=1, space="PSUM") as psA:
                pk0 = psA.tile([128, 512], F32, tag="pre", bufs=2, name="pk0")
                pq0 = psA.tile([128, 512], F32, tag="pre", bufs=2, name="pq0")
                for half in range(2):
                    for d in range(half * 4, half * 4 + 4):
                        nc.tensor.matmul(
                            pk0[:],
                            wk_sb[:, d * 128:(d + 1) * 128],
                            xkvC[0][:, d, :],
                            start=(d == 0), stop=(d == ND - 1),
                        )
                    for d in range(half * 4, half * 4 + 4):
                        nc.tensor.matmul(
                            pq0[:],
                            wq_sb[:, d * 128:(d + 1) * 128],
                            xq[0][:, d, :],
                            start=(d == 0), stop=(d == ND - 1),
                        )
                nc.vector.tensor_scalar_add(
                    kt_sb[:, 0:512], pk0[:], bk_sb[:, 0:1])
                nc.vector.tensor_scalar_add(
                    qt_sb[:, 0:512], pq0[:], bq_sb[:, 0:1])

            # ---- attention + fillers ----------------------------------
            with (
                tc.tile_pool(name="at", bufs=1) as at,
                tc.tile_pool(name="psAt", bufs=1, space="PSUM") as ps,
            ):
                # ------ filler closures --------------------------------
                def mk_vprime(j):
                    def emit():
                        pv = ps.tile([128, 512], F32, tag="fill", bufs=2,
                                     name=f"pv{j}")
                        qc, jr = divmod(j, 4)
                        for d in range(ND):
                            nc.tensor.matmul(
                                pv[:, 0:VW],
                                xkvC[qc][:, d, jr * 128:(jr + 1) * 128],
                                wvp_sb[:, d * VW:(d + 1) * VW],
                                start=(d == 0), stop=(d == ND - 1),
                            )
                        for h in range(HPG):
                            nc.vector.tensor_copy(
                                vp_sb[:, j * VW + h * 65:j * VW + h * 65 + 64],
                                pv[:, h * 65:h * 65 + 64])
                    return emit

                def mk_kt(p, qc):
                    # 4 pieces of 2 d-MMs each (spread over consecutive
                    # slots so the next scores are never pushed out by a
                    # long filler burst); the acc tile is shared via state
                    state = {}

                    def piece(k):
                        def emit():
                            if k == 0:
                                state["acc"] = ps.tile(
                                    [128, 512], F32, tag="fill", bufs=2,
                                    name=f"ktf{p}{qc}")
                            acc = state["acc"]
                            for d in range(4 * k, 4 * k + 4):
                                nc.tensor.matmul(
                                    acc[:],
                                    wk_sb[:, (p * ND + d) * 128:
                                          (p * ND + d + 1) * 128],
                                    xkvC[qc][:, d, :],
                                    start=(d == 0), stop=(d == ND - 1),
                                )
                            if k == 1:
                                nc.vector.tensor_scalar_add(
                                    kt_sb[:, p * SKV + qc * 512:
                                          p * SKV + (qc + 1) * 512],
                                    acc[:], bk_sb[:, p:p + 1])
                        return emit
                    return [piece(k) for k in range(2)]

                def mk_qt(p, qq):
                    state = {}

                    def piece(k):
                        def emit():
                            if k == 0:
                                state["acc"] = ps.tile(
                                    [128, 512], F32, tag="fill", bufs=2,
                                    name=f"qtf{p}{qq}")
                            acc = state["acc"]
                            for d in range(4 * k, 4 * k + 4):
                                nc.tensor.matmul(
                                    acc[:],
                                    wq_sb[:, (p * ND + d) * 128:
                                          (p * ND + d + 1) * 128],
                                    xq[qq][:, d, :],
                                    start=(d == 0), stop=(d == ND - 1),
                                )
                            if k == 1:
                                nc.vector.tensor_scalar_add(
                                    qt_sb[:, p * SQ + qq * 512:
                                          p * SQ + (qq + 1) * 512],
                                    acc[:], bq_sb[:, p:p + 1])
                        return emit
                    return [piece(k) for k in range(2)]

                def mk_outproj(s):
                    state = {}

                    def piece(n2):
                        def emit():
                            o_half = o_sbA if s < 8 else o_sbB
                            s8 = s % 8
                            if n2 == 0:
                                state["ob"] = at.tile(
                                    [128, 1024], BF16, tag="ob", bufs=3,
                                    name=f"ob{s}")
                            ob = state["ob"]
                            po = ps.tile([128, 512], F32, tag="fill", bufs=2,
                                         name=f"po{s}{n2}")
                            for tt in range(2):
                                nc.tensor.matmul(
                                    po[:],
                                    o_half[:, tt * 1024 + s8 * 128:
                                           tt * 1024 + (s8 + 1) * 128],
                                    wo_sb[:, tt * D + n2 * 512:
                                          tt * D + n2 * 512 + 512],
                                    start=(tt == 0), stop=(tt == 1),
                                )
                            nc.vector.tensor_copy(
                                ob[:, n2 * 512:(n2 + 1) * 512], po[:])
                            if n2 == 1:
                                nc.sync.dma_start(
                                    out=out_d[s * 128:(s + 1) * 128, :],
                                    in_=ob[:])
                        return emit
                    return [piece(0), piece(1)]

                # fillers per block, keyed by slot index (emission order)
                blocks = [(0, 0), (0, 1), (0, 2), (0, 3),
                          (1, 0), (1, 1), (1, 2), (1, 3)]
                fill = {b: {} for b in range(8)}

                def add_fill(b, slot, fn):
                    fill[b].setdefault(slot, []).append(fn)

                for j in range(NJ):
                    add_fill(0, j, mk_vprime(j))
                # remaining KT(p0) chunks just-in-time, paced by the xkv
                # column-block DMAs (chunk qc gates scores j>=4qc)
                def add_pieces(b, slot, pieces):
                    for k, fn in enumerate(pieces):
                        add_fill(b, slot + k, fn)

                add_pieces(0, 2, mk_kt(0, 1))
                add_pieces(0, 6, mk_kt(0, 2))
                add_pieces(0, 10, mk_kt(0, 3))
                add_pieces(0, 11, mk_qt(0, 1))
                # each QT/KT a full block ahead of its consumer so the
                # DVE bias-add is never on the critical path
                add_pieces(1, 0, mk_qt(0, 2))
                add_pieces(1, 8, mk_qt(0, 3))
                add_pieces(2, 0, mk_kt(1, 0))
                add_pieces(2, 4, mk_kt(1, 1))
                add_pieces(2, 8, mk_kt(1, 2))
                add_pieces(2, 12, mk_kt(1, 3))
                add_pieces(3, 0, mk_qt(1, 0))
                add_pieces(3, 4, mk_qt(1, 1))
                add_pieces(3, 8, mk_qt(1, 2))
                add_pieces(3, 12, mk_qt(1, 3))
                for i, s in enumerate(range(0, 4)):
                    add_pieces(5, 2 + 3 * i, mk_outproj(s))
                for i, s in enumerate(range(4, 8)):
                    add_pieces(6, 2 + 3 * i, mk_outproj(s))
                for i, s in enumerate(range(8, 12)):
                    add_pieces(7, 1 + 2 * i, mk_outproj(s))

                def emit_norm(t, qq, hp, o_ps):
                    rs = at.tile([1, 512], F32, tag="rs", bufs=4,
                                 name=f"rs{t}{qq}{hp}")
                    nc.vector.tensor_copy(rs[:], o_ps[64:65, :])
                    rcp = at.tile([1, 512], F32, tag="rcp", bufs=4,
                                  name=f"rcp{t}{qq}{hp}")
                    nc.vector.reciprocal_approx_fast(rcp[:], rs[:])
                    bcs = at.tile([64, 512], F32, tag="bcs", bufs=4,
                                  name=f"bcs{t}{qq}{hp}")
                    nc.gpsimd.partition_broadcast(bcs[:], rcp[:], channels=64)
                    o_half = o_sbA if qq < 2 else o_sbB
                    col = t * 1024 + (qq % 2) * 512
                    nc.vector.tensor_tensor(
                        out=o_half[hp * 64:(hp + 1) * 64, col:col + 512],
                        in0=o_ps[0:64, :], in1=bcs[:], op=MUL)

                # ------ the attention stream ---------------------------
                for b, (t, qq) in enumerate(blocks):
                    o_ps = {}
                    for hp in range(2):
                        o_ps[hp] = ps.tile([128, 512], F32, tag="o_ps",
                                           bufs=2, name=f"o_ps{t}{qq}{hp}")
                    p_ts = {}
                    for j in range(NJ + 2):
                        if j < NJ:
                            st = ps.tile([128, 1024], F32, tag="st2", bufs=2,
                                         name=f"st{t}{qq}{j}")
                            for hp in range(2):
                                nc.tensor.matmul(
                                    st[:, hp * 512:(hp + 1) * 512],
                                    kt_sb[hp * 64:(hp + 1) * 64,
                                          t * SKV + j * 128:
                                          t * SKV + (j + 1) * 128],
                                    qt_sb[hp * 64:(hp + 1) * 64,
                                          t * SQ + qq * 512:
                                          t * SQ + (qq + 1) * 512],
                                    start=True, stop=True,
                                )
                            p_t = at.tile([128, 1024], BF16, tag="pt",
                                          bufs=12, name=f"pt{t}{qq}{j}")
                            nc.scalar.activation(p_t[:], st[:],
                                                 AF.Exp, scale=scale)
                            p_ts[j] = p_t
                        for fn in fill[b].get(j, []):
                            fn()
                        if j >= 2:
                            ja = j - 2
                            p_t = p_ts.pop(ja)
                            for hp in range(2):
                                h = 2 * t + hp
                                nc.tensor.matmul(
                                    o_ps[hp][:],
                                    vp_sb[:, ja * VW + h * 65:
                                          ja * VW + h * 65 + 128],
                                    p_t[:, hp * 512:(hp + 1) * 512],
                                    start=(ja == 0), stop=(ja == NJ - 1),
                                )
                    for hp in range(2):
                        emit_norm(t, qq, hp, o_ps[hp])

                # tail: last qq group's output projection
                for s in range(12, 16):
                    for fn in mk_outproj(s):
                        fn()

    nc.compile()
    return nc


def build_in_maps(inputs):
    query_input = np.asarray(inputs["query_input"], dtype=np.float32)
    kv_input = np.asarray(inputs["kv_input"], dtype=np.float32)
    Wq = np.asarray(inputs["Wq"], dtype=np.float32)
    bq = np.asarray(inputs["bq"], dtype=np.float32)
    Wkv = np.asarray(inputs["Wkv"], dtype=np.float32)
    bkv = np.asarray(inputs["bkv"], dtype=np.float32)
    Wo = np.asarray(inputs["Wo"], dtype=np.float32)

    Wk = Wkv[:, :D]
    Wv = Wkv[:, D:]
    bk = bkv[:D]

    def tile_x(xt):
        # [D, SQ] -> [128, 4(qcol), 8(d), 512]
        return np.ascontiguousarray(
            xt.reshape(8, 128, 4, 512).transpose(1, 2, 0, 3)).astype(BF16NP)

    def tile_w(w):
        # [D, N] -> [128, 8(d), N]
        n = w.shape[1]
        return np.ascontiguousarray(
            w.reshape(8, 128, n).transpose(1, 0, 2)).astype(BF16NP)

    def tile_w_p(w):
        # [D, 256] -> [128, 2(pair), 8(d), 128]
        return np.ascontiguousarray(
            w.reshape(8, 128, 2, 128).transpose(1, 2, 0, 3)).astype(BF16NP)

    xT = [tile_x(query_input[b].T) for b in range(B)]
    kvT = [tile_x(kv_input[b].T) for b in range(B)]

    in_maps = []
    for c in range(N_CORES):
        b, g = divmod(c, G)
        c0 = g * GC
        wvp = np.zeros((D, HPG * 65), np.float32)
        for h in range(HPG):
                wvp[:, h * 65:h * 65 + 64] = Wv[:, c0 + h * DH:c0 + (h + 1) * DH]
        bq2 = bq[c0:c0 + GC].reshape(2, 128).T.copy()
        bk2 = bk[c0:c0 + GC].reshape(2, 128).T.copy()
        wo_t = np.ascontiguousarray(
            Wo[c0:c0 + GC, :].reshape(2, 128, D).transpose(1, 0, 2))
        in_maps.append({
                "xqT": xT[b],
                "xkvT": kvT[b],
                "wq": tile_w_p(Wq[:, c0:c0 + GC]),
                "wk": tile_w_p(Wk[:, c0:c0 + GC]),
                "wvp": tile_w(wvp),
                "wo": wo_t.astype(BF16NP),
                "bq2": np.ascontiguousarray(bq2),
                "bk2": np.ascontiguousarray(bk2),
        })
    return in_maps


def kernel(query_input, kv_input, Wq, bq, Wkv, bkv, Wo, bo):
    global _nc_cache
    from concourse import bass_utils

    if _nc_cache is None:
        _nc_cache = _build_nc()
    nc = _nc_cache

    Wkv = np.asarray(Wkv, dtype=np.float32)
    Wo = np.asarray(Wo, dtype=np.float32)
    bo = np.asarray(bo, dtype=np.float32)
    bv = np.asarray(bkv, np.float32)[D:]

    in_maps = build_in_maps(dict(
        query_input=query_input, kv_input=kv_input, Wq=Wq, bq=bq,
        Wkv=Wkv, bkv=bkv, Wo=Wo))

    res = bass_utils.run_bass_kernel_spmd(nc, in_maps,
                                          core_ids=list(range(N_CORES)))

    # gather: sum the 4 head-group partials per batch; add biases the device
    # left out (bo, and bv which passes through Wo since softmax rows sum to 1)
    tail = bv @ Wo + bo
    out = np.empty((B, SQ, D), np.float32)
    for b in range(B):
        acc = res.results[b * G + 0]["out_p"].astype(np.float32)
        for g in range(1, G):
                acc = acc + res.results[b * G + g]["out_p"].astype(np.float32)
        out[b] = acc + tail[None, :]
    return out
